# revision 2
# baseline (speedup 1.0000x reference)
"""AttentionJacobian kernel for 8 TRN2 NeuronCores.

TimelineSim per-core: 233.6 us (prior bf16 LP-split baseline: 256.1 us,
staged naive: 372.2 us).  Measured device rel err: 1.59e-2 (tol 2e-2).

J[b,q] = SCALE * ( V^T diag(a_q) K  -  o_q w_q^T ),  a = softmax(SCALE Q K^T)

Data-parallel over batch: 16 batches -> 2 per core.  sk tiles use
(k-major, j-minor) column order so the DVE pair-ops hit the 2x_1p mode;
matmul moving APs read them (j, k)-ordered so psum stays j-contiguous for
the rank-1 term2 closes and the evacuation.

Per (group g of 8 q's, chunk-pair p of 2 seq chunks), two pair kinds:
  KREP (11 pairs): one DVE tensor_tensor op sk[n,s,k,j] = ATb[n,c,qj]*Krep
                   (Krep = K replicated x8 along j, streamed from DRAM on
                   the gpsimd DMA queue); 2 bf16 matmuls per chunk.
  FP8  (5 pairs):  16 per-q tensor_scalar/activation ops (LP-scheduled on
                   DVE/Act/Pool) writing e4m3 pair tiles; 4 DoubleRow
                   strips x {V_hi, V_lo} stationaries (V hi/lo split
                   compensates fp8 quantization of V).
A global c=1024 is folded into the softmax normalizer (rzb = c/Z) so fp8
values sit in e4m3's normal range; the evacuation applies SCALE/c.
"""

import sys

for p in ("/opt/trn_rl_repo",):
    if p not in sys.path:
        sys.path.append(p)

import numpy as np
import ml_dtypes

import concourse.bass as bass
import concourse.bacc as bacc
import concourse.tile as tile
from concourse import mybir
from concourse.bass_utils import run_bass_kernel_spmd

N_CORES = 8
BATCH = 16
NQ = 64
SEQ = 4096
D = 128
BPC = BATCH // N_CORES        # batches per core = 2
C = SEQ // 128                # 32 contraction chunks
NP = C // 2                   # 16 chunk pairs
QG = 8                        # q per output group
NG = NQ // QG                 # 8 groups
SCALE = float(D) ** -0.5
CSC = 1024.0                  # fp8 range scale folded into rzb

import os as _os
_nfp8 = int(_os.environ.get("V2_NFP8", "5"))
FP8POS = (1, 4, 7, 10, 13)[:_nfp8]         # fp8 pairs (env-tunable)
KREPPOS = tuple(p for p in range(NP) if p not in FP8POS)  # 11 krep pairs
KREP_IDX = {p: i for i, p in enumerate(KREPPOS)}
FP8_IDX = {p: i for i, p in enumerate(FP8POS)}

F32 = mybir.dt.float32
BF16 = mybir.dt.bfloat16
FP8 = mybir.dt.float8e4
AF = mybir.ActivationFunctionType
ALU = mybir.AluOpType

_CACHED = {}


def _build():
    nc = bacc.Bacc("TRN2", target_bir_lowering=False, debug=False,
                   num_devices=N_CORES)

    kvb = nc.dram_tensor("kvb", [BPC, C, 128, 256], BF16, kind="ExternalInput").ap()
    krep = nc.dram_tensor("krep", [BPC, len(KREPPOS), 128, 2048], BF16,
                          kind="ExternalInput").ap()
    v8a = nc.dram_tensor("v8a", [BPC, max(1, 2 * len(FP8POS)), 128, 256],
                         FP8, kind="ExternalInput").ap()
    kt = nc.dram_tensor("kt", [BPC, 128, SEQ], BF16, kind="ExternalInput").ap()
    qt = nc.dram_tensor("qt", [BPC, 128, NQ], BF16, kind="ExternalInput").ap()
    crow = nc.dram_tensor("crow", [BPC, 1, 128], F32, kind="ExternalInput").ap()
    out = nc.dram_tensor("out", [BPC, NQ, D, D], F32, kind="ExternalOutput").ap()

    with tile.TileContext(nc) as tc:
        with (
            tc.tile_pool(name="const", bufs=1) as constp,
            tc.tile_pool(name="kv", bufs=2) as kvp,
            tc.tile_pool(name="krpA", bufs=2) as krpA,
            tc.tile_pool(name="krpB", bufs=1) as krpB,
            tc.tile_pool(name="v8p", bufs=2) as v8p,
            tc.tile_pool(name="ktp", bufs=1) as ktp,
            tc.tile_pool(name="qtp", bufs=2) as qtp,
            tc.tile_pool(name="ep", bufs=2) as ep,
            tc.tile_pool(name="rzp", bufs=2) as rzp,
            tc.tile_pool(name="atp", bufs=2) as atp,
            tc.tile_pool(name="owp", bufs=2) as owp,
            tc.tile_pool(name="skp", bufs=6) as skp,
            tc.tile_pool(name="sk8p", bufs=4) as sk8p,
            tc.tile_pool(name="jsbp", bufs=2) as jsbp,
            tc.tile_pool(name="owdram", bufs=2, space="DRAM") as owdp,
            tc.tile_pool(name="psj", bufs=2, space="PSUM") as psjp,
            tc.tile_pool(name="pss", bufs=1, space="PSUM") as pssp,
            tc.tile_pool(name="psmall", bufs=1, space="PSUM") as psmp,
        ):
            onescol = constp.tile([128, 1], BF16)
            nc.vector.memset(onescol[:, :], 1.0)
            onesrowC = constp.tile([1, 128], F32)
            nc.vector.memset(onesrowC[:, :], CSC)
            onesf1 = constp.tile([1, 1], F32)
            nc.vector.memset(onesf1[:, :], 1.0)
            warm = constp.tile([128, 512], BF16)
            nc.vector.memset(warm[:, :], 0.0)

            # static LP counters for per-q fp8 op placement (ns each)
            eng_cost = {"d": 127.0, "a": 292.0, "p": 273.0}
            eng_load = {"d": 0.0, "a": 0.0, "p": 0.0}
            # per-group fixed load of each engine besides per-q fp8 ops
            eng_bias = {"d": 12600.0, "a": 1400.0, "p": 700.0}

            def pick_engine(allowed="dap"):
                best, bestt = None, None
                for e in allowed:
                    t = eng_bias[e] + eng_load[e] + eng_cost[e]
                    if bestt is None or t < bestt:
                        best, bestt = e, t
                eng_load[best] += eng_cost[best]
                return best

            def head(b, st):
                """Per-batch prologue (yield between pieces)."""
                QT = qtp.tile([128, NQ], BF16, tag="qt")
                nc.sync.dma_start(QT[:, :], qt[b])
                crt = qtp.tile([1, 128], F32, tag="crow")
                nc.sync.dma_start(crt[:, :], crow[b])
                st["crt"] = crt
                KT = ktp.tile([128, SEQ], BF16, tag="kt")
                for kc in range(4):
                    nc.sync.dma_start(KT[:, kc * 1024:(kc + 1) * 1024],
                                      kt[b][:, kc * 1024:(kc + 1) * 1024])
                KVt = [kvp.tile([128, 8 * 256], BF16, tag=f"kv{i}",
                                name=f"kvt{i}") for i in range(4)]
                for i in range(4):
                    nc.sync.dma_start(
                        KVt[i][:, :].rearrange("p (c j) -> p c j", j=256),
                        kvb[b, i * 8:(i + 1) * 8].rearrange("c n j -> n c j"))
                # coalesced loads: 1 v8 DMA + 3 krep DMAs (shared HWDGE
                # is a serial resource; fewer/earlier configs -> short head)
                V8a = v8p.tile([128, max(1, 2 * len(FP8POS)) * 256], FP8,
                               tag="v8a")
                if FP8POS:
                    nc.sync.dma_start(
                        V8a[:, :].rearrange("p (i m) -> p i m", m=256),
                        v8a[b].rearrange("i n m -> n i m"))
                nkr = len(KREPPOS)
                splits = []
                lo = 0
                while lo < nkr:
                    hi = min(lo + 4, nkr)
                    splits.append((lo, hi))
                    lo = hi
                KRT = [(krpA if t == 0 else krpB)
                       .tile([128, (hi - lo) * 2048], BF16, tag=f"kr{t}",
                             name=f"krt{t}")
                       for t, (lo, hi) in enumerate(splits)]
                for t, (lo, hi) in enumerate(splits):
                    nc.sync.dma_start(
                        KRT[t][:, :].rearrange("p (i m) -> p i m", m=2048),
                        krep[b, lo:hi].rearrange("i n m -> n i m"))
                st["KVt"], st["KRT"], st["V8a"] = KVt, KRT, V8a
                E = ep.tile([128, C * NQ], BF16, tag="e")
                st["E"] = E
                yield

                ps_banks = []
                ps_sm = psmp.tile([128, 512], F32, tag="small")
                st["ps_sm"] = ps_sm
                ps_zw = ps_sm[0:1, 0:512]
                if b == 0:
                    # p-state warmup: keep PE busy from t~0 so scores run at
                    # full clock (ramp needs ~3us of continuous execution);
                    # results are discarded by Zwide's start=True reset
                    for _ in range(6):
                        nc.tensor.matmul(ps_zw, onescol[:, :], warm[:, :],
                                         start=True, stop=True,
                                         skip_group_check=True)
                for cs in range(C // 8):
                    ps_s = pssp.tile([128, 8 * NQ], F32, tag=f"scores{cs % 3}")
                    for c8 in range(8):
                        c = cs * 8 + c8
                        nc.tensor.matmul(ps_s[:, c8 * NQ:(c8 + 1) * NQ],
                                         KT[:, c * 128:(c + 1) * 128],
                                         QT[:, :], start=True, stop=True)
                    ps_banks.append(ps_s)
                    if cs >= 1:
                        prev = ps_banks[cs - 1]
                        nc.scalar.activation(
                            E[:, (cs - 1) * 8 * NQ:cs * 8 * NQ],
                            prev[:, :], AF.Exp, bias=0.0, scale=SCALE)
                    if cs >= 2:
                        i = cs - 2
                        nc.tensor.matmul(ps_zw, onescol[:, :],
                                         E[:, i * 512:(i + 1) * 512],
                                         start=(i == 0), stop=False,
                                         skip_group_check=True)
                    if cs == 1:
                        yield
                nc.scalar.activation(E[:, 3 * 8 * NQ:4 * 8 * NQ],
                                     ps_banks[3][:, :], AF.Exp, bias=0.0,
                                     scale=SCALE)
                for i in (2, 3):
                    nc.tensor.matmul(ps_zw, onescol[:, :],
                                     E[:, i * 512:(i + 1) * 512],
                                     start=False, stop=(i == 3),
                                     skip_group_check=True)

                zrow = rzp.tile([1, NQ], F32, tag="zrow")
                nc.vector.tensor_reduce(
                    zrow[:, :], ps_zw.rearrange("p (c q) -> p q c", q=NQ),
                    mybir.AxisListType.X, ALU.add)
                rz = rzp.tile([1, NQ], F32, tag="rz")
                nc.vector.reciprocal(rz[:, :], zrow[:, :])
                st["rz"] = rz
                ps_rzb = ps_sm[:, NQ:2 * NQ]
                nc.tensor.matmul(ps_rzb, onesrowC[:, :], rz[:, :],
                                 start=True, stop=True)
                rzb = rzp.tile([128, NQ], F32, tag="rzbsb")
                nc.scalar.copy(rzb[:, :], ps_rzb)            # c/Z  (f32)
                rzbb = rzp.tile([128, NQ], BF16, tag="rzbbf")
                nc.scalar.copy(rzbb[:, :], ps_rzb)           # c/Z  (bf16)
                # fp8-path normalizer gamma_b*c/Z (per-batch range control)
                ps_rzb8 = ps_sm[:, 192:256]
                nc.tensor.matmul(ps_rzb8, st["crt"][:, :], rz[:, :],
                                 start=True, stop=True)
                rzb8 = rzp.tile([128, NQ], F32, tag="rzb8sb")
                nc.scalar.copy(rzb8[:, :], ps_rzb8)
                ATb = atp.tile([128, C * NQ], BF16, tag="atb")
                ATf8 = atp.tile([128, max(1, 2 * len(FP8POS)) * NQ], F32,
                                tag="atf8")
                st["ATb"], st["ATf8"] = ATb, ATf8

                def emit_atf(g):
                    s = g * QG
                    # bf16 scalars for the DVE pair-ops (2x mode needs bf16)
                    nc.vector.tensor_mul(
                        ATb[:, :].rearrange("p (c q) -> p c q", q=NQ)[:, :, s:s + QG],
                        E[:, :].rearrange("p (c q) -> p c q", q=NQ)[:, :, s:s + QG],
                        rzbb[:, s:s + QG].unsqueeze(1).broadcast_to((128, C, QG)),
                    )
                    # f32 scalars for fp8 per-q ops (scale APs must be f32)
                    ev = E[:, :].rearrange("p (c q) -> p c q", q=NQ)
                    av = (ATf8[:, :].rearrange("p (c q) -> p c q", q=NQ)
                          if FP8POS else None)
                    for i, pp in enumerate(FP8POS):
                        nc.gpsimd.tensor_mul(
                            av[:, 2 * i:2 * i + 2, s:s + QG],
                            ev[:, 2 * pp:2 * pp + 2, s:s + QG],
                            rzb8[:, s:s + QG].unsqueeze(1)
                            .broadcast_to((128, 2, QG)),
                        )

                st["emit_atf"] = emit_atf
                emit_atf(0)
                ow_prologue(b, st)
                yield

            rank1_pending = []
            evac_pending = []

            def flush_rank1():
                bp, gp, ps_prev, stp = rank1_pending.pop(0)
                owf = stp["owf"]
                bp_part = 32 * (gp % 3)
                cb = (gp // 3) * 2048
                for j in range(QG):
                    nc.tensor.matmul(
                        ps_prev[:, j * 128:(j + 1) * 128],
                        owf[bp_part:bp_part + 1, cb + j * 256:cb + j * 256 + 128],
                        owf[bp_part:bp_part + 1,
                            cb + j * 256 + 128:cb + (j + 1) * 256],
                        start=False, stop=True, skip_group_check=True)
                evac_pending.append((bp, gp, ps_prev))

            def flush_evac():
                bp, gp, ps_prev = evac_pending.pop(0)
                jsb = jsbp.tile([128, QG * 128], F32, tag="jsb")
                if bp == BPC - 1 and gp == NG - 1:
                    h = QG // 4
                    for s in range(4):
                        nc.scalar.activation(
                            jsb[:, s * h * 128:(s + 1) * h * 128],
                            ps_prev[:, s * h * 128:(s + 1) * h * 128],
                            AF.Copy, bias=0.0, scale=SCALE / CSC)
                        nc.sync.dma_start(
                            out[bp, gp * QG + s * h:gp * QG + (s + 1) * h]
                            .rearrange("j v k -> v j k"),
                            jsb[:, s * h * 128:(s + 1) * h * 128]
                            .rearrange("v (j k) -> v j k", k=128),
                        )
                    return
                nc.scalar.activation(jsb[:, :], ps_prev[:, :],
                                     AF.Copy, bias=0.0, scale=SCALE / CSC)
                nc.sync.dma_start(
                    out[bp, gp * QG:(gp + 1) * QG].rearrange("j v k -> v j k"),
                    jsb[:, :].rearrange("v (j k) -> v j k", k=128),
                )

            def ow_prologue(b, st):
                KVt, E, ps_sm, rz = st["KVt"], st["E"], st["ps_sm"], st["rz"]
                ps_rzq = ps_sm[0:NQ, 128:129]
                nc.tensor.matmul(ps_rzq, rz[:, :], onesf1[:, :],
                                 is_transpose=True, start=True, stop=True)
                rq = rzp.tile([NQ, 1], F32, tag="rqsb")
                nc.vector.tensor_copy(rq[:, :], ps_rzq)
                m_o = rzp.tile([NQ, 1], F32, tag="mo")
                nc.vector.scalar_tensor_tensor(m_o[:, :], rq[:, :], -CSC,
                                               rq[:, :], ALU.mult, ALU.mult)
                ps_ow = ps_sm[0:NQ, 256:512]
                for c in range(C):
                    kvc = KVt[c // 8][:, (c % 8) * 256:(c % 8 + 1) * 256]
                    nc.tensor.matmul(ps_ow, E[:, c * NQ:(c + 1) * NQ], kvc,
                                     start=(c == 0), stop=(c == C - 1))
                owsb = owp.tile([NQ, 256], BF16, tag="owsb")
                nc.scalar.mul(owsb[:, 0:128], ps_ow[:, 0:128], m_o[:, 0:1])
                nc.scalar.copy(owsb[:, 128:256], ps_ow[:, 128:256])
                owd = owdp.tile([NQ, 256], BF16, tag="owd")
                nc.sync.dma_start(owd[:, :], owsb[:, :])
                # scatter each q-group's (o|w) rows to partition 32*(g%3),
                # column slot g//3, so rank-1 lhsT sits on a PE tile boundary
                # (base_partition() only allows 0/32/64)
                owf = owp.tile([128, 3 * 2048], BF16, tag="owf")
                for g in range(NG):
                    nc.sync.dma_start(
                        owf[32 * (g % 3):32 * (g % 3) + 1,
                            (g // 3) * 2048:(g // 3 + 1) * 2048],
                        owd[g * QG:(g + 1) * QG, :]
                        .rearrange("q m -> (q m)").unsqueeze(0))
                st["owf"] = owf

            def emit_krep_pair(st, g, p, first=False):
                """One DVE tensor_tensor op -> sk[n, s, k, j] for both chunks
                of pair p, then 2 bf16 matmuls per chunk (rhs read (j,k))."""
                ATb, KVt, KRT = st["ATb"], st["KVt"], st["KRT"]
                ps_j = st["ps_j"]
                sk = skp.tile([128, 2048], BF16, tag="sk")
                atv = (ATb[:, :].rearrange("p (c q) -> p c q", q=NQ)
                       [:, 2 * p:2 * p + 2, g * QG:g * QG + QG]
                       .unsqueeze(2).broadcast_to((128, 2, 128, QG)))
                ki = KREP_IDX[p]
                krt = KRT[ki // 4]
                ko = (ki % 4) * 2048
                nc.vector.tensor_mul(
                    sk[:, :].rearrange("p (s k j) -> p s k j", k=128, j=QG),
                    krt[:, ko:ko + 2048].rearrange("p (s k j) -> p s k j",
                                                   k=128, j=QG),
                    atv)
                skjk = sk[:, :].rearrange("p (s k j) -> p s j k", k=128, j=QG)
                for s in range(2):
                    c = 2 * p + s
                    kvb_c = KVt[c // 8]
                    co = (c % 8) * 256
                    for h in range(2):
                        nc.tensor.matmul(
                            ps_j[:, h * 512:(h + 1) * 512],
                            kvb_c[:, co:co + 128],
                            skjk[:, s, h * 4:(h + 1) * 4, :],
                            start=(first and s == 0), stop=False,
                            skip_group_check=True)

            def emit_fp8_pair(st, g, p, tail, first=False):
                """16 per-q fp8 ops (engine-scheduled) + 4 DoubleRow strips
                x {V_hi, V_lo}."""
                ATf8, KVt, V8a = st["ATf8"], st["KVt"], st["V8a"]
                ps_j = st["ps_j"]
                i8 = FP8_IDX[p]
                # sk8 layout (s, j, k): per-q writes are contiguous 128-runs
                # and each DoubleRow strip is a plain 3-D [p, 2, 256] slice
                sk8 = sk8p.tile([128, 2048], FP8, tag="sk8")
                skv = sk8[:, :].rearrange("p (s j k) -> p s j k", k=128, j=QG)
                for s in range(2):
                    c = 2 * p + s
                    kvb_c = KVt[c // 8]
                    kslice = kvb_c[:, (c % 8) * 256 + 128:(c % 8) * 256 + 256]
                    for j in range(QG):
                        q = g * QG + j
                        acol = ATf8[:, (2 * i8 + s) * NQ + q:
                                    (2 * i8 + s) * NQ + q + 1]
                        dst = skv[:, s, j, :]
                        e = pick_engine("da" if (tail and p >= 10) else "dap")
                        if e == "d":
                            nc.vector.tensor_scalar_mul(dst, kslice, acol)
                        elif e == "a":
                            nc.scalar.mul(dst, kslice, acol)
                        else:
                            nc.gpsimd.tensor_scalar_mul(dst, kslice, acol)
                rhsv = sk8[:, :].rearrange("p (s m) -> p s m", s=2)
                v8v = V8a[:, :].rearrange("p (i s v) -> p i s v", s=2, v=128)
                vhi = v8v[:, 2 * i8]
                vlo = v8v[:, 2 * i8 + 1]
                for h in range(4):
                    rhs = rhsv[:, :, h * 256:(h + 1) * 256]
                    for vi, vst in enumerate((vhi, vlo)):
                        nc.tensor.matmul(
                            ps_j[:, h * 256:(h + 1) * 256],
                            vst, rhs,
                            start=(first and vi == 0), stop=False,
                            perf_mode=mybir.MatmulPerfMode.DoubleRow,
                            skip_group_check=True)

            def term1(b, st):
                """Per-batch main loop; yields after each of NG groups."""
                for g in range(NG):
                    ps_j = psjp.tile([128, QG * 128], F32, tag="j")
                    st["ps_j"] = ps_j
                    tail = (b == BPC - 1 and g == NG - 1)
                    for i, p in enumerate(range(NP)):
                        if i == 8 and g + 1 < NG:
                            st["emit_atf"](g + 1)
                        if i == (2 if tail else 12) and rank1_pending:
                            flush_rank1()
                        if i == (5 if tail else 14) and evac_pending:
                            flush_evac()
                        if p in KREP_IDX:
                            emit_krep_pair(st, g, p, first=(i == 0))
                        else:
                            emit_fp8_pair(st, g, p, tail, first=(i == 0))
                    for e in eng_load:
                        eng_load[e] = 0.0
                    rank1_pending.append((b, g, ps_j, st))
                    yield

            states = [{} for _ in range(BPC)]
            heads = [head(b, states[b]) for b in range(BPC)]
            terms = [term1(b, states[b]) for b in range(BPC)]
            for _ in heads[0]:
                pass
            for b in range(BPC):
                nxt = heads[b + 1] if b + 1 < BPC else None
                for g in range(NG):
                    next(terms[b], None)
                    if nxt is not None:
                        next(nxt, None)
            while rank1_pending:
                flush_rank1()
            while evac_pending:
                flush_evac()

    nc.compile()
    return nc


def _get_nc():
    if "nc" not in _CACHED:
        _CACHED["nc"] = _build()
    return _CACHED["nc"]


def _prep_core_inputs(query, keys, values, i):
    s = slice(i * BPC, (i + 1) * BPC)
    K = np.ascontiguousarray(keys[s])     # (2, 4096, 128) f32
    V = np.ascontiguousarray(values[s])
    Q = np.ascontiguousarray(query[s])    # (2, 64, 128) f32
    kvb = np.empty((BPC, C, 128, 256), dtype=ml_dtypes.bfloat16)
    kvb[:, :, :, 0:128] = V.reshape(BPC, C, 128, 128)
    kvb[:, :, :, 128:256] = K.reshape(BPC, C, 128, 128)
    # krep[b, i, n, (s, k, j)] = K[b, (2*KREPPOS[i]+s)*128+n, k]  (j x8)
    Kc = K.reshape(BPC, NP, 2, 128, 128).astype(ml_dtypes.bfloat16)
    kr = Kc[:, KREPPOS]                               # (2, 11, 2, 128n, 128k)
    kr = kr.transpose(0, 1, 3, 2, 4)                  # (2, 11, 128n, 2s, 128k)
    krep = np.broadcast_to(kr[..., None],
                           (BPC, len(KREPPOS), 128, 2, 128, QG))
    krep = np.ascontiguousarray(krep).reshape(BPC, len(KREPPOS), 128, 2048)
    # per-batch fp8 range scale: keep max |gamma*c*a*K| under ~192
    sc = np.einsum('bqd,bnd->bqn', Q, K) * SCALE
    sc -= sc.max(axis=2, keepdims=True)
    e = np.exp(sc)
    amax = (e / e.sum(axis=2, keepdims=True)).max(axis=(1, 2))   # (BPC,)
    kmax = np.abs(K).max(axis=(1, 2))
    gamma = np.minimum(1.0, 192.0 / (CSC * amax * kmax)).astype(np.float32)
    crow = np.broadcast_to((gamma * CSC)[:, None, None],
                           (BPC, 1, 128)).astype(np.float32)
    crow = np.ascontiguousarray(crow)
    # V fp8 hi/lo for fp8 pairs (scaled by 1/gamma): v8[b, i, n, (s, v)]
    Vc = V.reshape(BPC, NP, 2, 128, 128) / gamma[:, None, None, None, None]
    v8 = Vc[:, FP8POS]                                # (2, 5, 2, 128n, 128v)
    v8 = np.ascontiguousarray(v8.transpose(0, 1, 3, 2, 4))  # (2,5,128n,2s,128v)
    v8hq = v8.astype(ml_dtypes.float8_e4m3)
    v8lq = (v8 - v8hq.astype(np.float32)).astype(ml_dtypes.float8_e4m3)
    v8all = np.zeros((BPC, max(1, 2 * len(FP8POS)), 128, 256),
                     dtype=ml_dtypes.float8_e4m3)
    if FP8POS:
        v8all[:, 0::2] = v8hq.reshape(BPC, len(FP8POS), 128, 256)
        v8all[:, 1::2] = v8lq.reshape(BPC, len(FP8POS), 128, 256)
    kt = np.ascontiguousarray(K.transpose(0, 2, 1)).astype(ml_dtypes.bfloat16)
    qt = np.ascontiguousarray(Q.transpose(0, 2, 1)).astype(ml_dtypes.bfloat16)
    return {"kvb": kvb, "krep": krep, "v8a": v8all, "kt": kt, "qt": qt,
            "crow": crow}


def _get_runner():
    """Build the jitted shard_map executable once and reuse it across calls."""
    if "runner" in _CACHED:
        return _CACHED["runner"]
    import jax
    from jax.sharding import Mesh, PartitionSpec
    try:
        from jax import shard_map
    except ImportError:
        from jax.experimental.shard_map import shard_map
    from concourse import bass2jax

    nc = _get_nc()
    bass2jax.install_neuronx_cc_hook()
    partition_name = (nc.partition_id_tensor.name
                      if nc.partition_id_tensor else None)
    in_names, out_names, out_avals, out_shapes = [], [], [], []
    for alloc in nc.m.functions[0].allocations:
        if not isinstance(alloc, mybir.MemoryLocationSet):
            continue
        name = alloc.memorylocations[0].name
        if alloc.kind == "ExternalInput":
            if name != partition_name:
                in_names.append(name)
        elif alloc.kind == "ExternalOutput":
            out_names.append(name)
            shape = tuple(alloc.tensor_shape)
            dtype = mybir.dt.np(alloc.dtype)
            out_avals.append(jax.core.ShapedArray(shape, dtype))
            out_shapes.append((shape, dtype))
    n_params = len(in_names)
    n_outs = len(out_avals)
    all_names = in_names + out_names
    if partition_name is not None:
        all_names.append(partition_name)
    donate = tuple(range(n_params, n_params + n_outs))

    def _body(*args):
        operands = list(args)
        if partition_name is not None:
            operands.append(bass2jax.partition_id_tensor())
        outs = bass2jax._bass_exec_p.bind(
            *operands, out_avals=tuple(out_avals), in_names=tuple(all_names),
            out_names=tuple(out_names), lowering_input_output_aliases=(),
            sim_require_finite=True, sim_require_nnan=True, nc=nc)
        return tuple(outs)

    devices = jax.devices()[:N_CORES]
    mesh = Mesh(np.asarray(devices), ("core",))
    sharded = jax.jit(
        shard_map(_body, mesh=mesh,
                  in_specs=(PartitionSpec("core"),) * (n_params + n_outs),
                  out_specs=(PartitionSpec("core"),) * n_outs,
                  check_rep=False),
        donate_argnums=donate, keep_unused=True)

    def run(in_maps):
        concat_in = [
            np.concatenate([np.asarray(in_maps[c][n]) for c in range(N_CORES)],
                           axis=0)
            for n in in_names]
        concat_zeros = [
            np.zeros((N_CORES * s[0], *s[1:]), dt) for s, dt in out_shapes]
        out_arrs = sharded(*concat_in, *concat_zeros)
        i = out_names.index("out")
        shape = out_shapes[i][0]
        return np.asarray(out_arrs[i]).reshape(N_CORES * shape[0], *shape[1:])

    _CACHED["runner"] = run
    return run


def kernel(query, keys, values):
    query = np.asarray(query, dtype=np.float32)
    keys = np.asarray(keys, dtype=np.float32)
    values = np.asarray(values, dtype=np.float32)
    in_maps = [_prep_core_inputs(query, keys, values, i) for i in range(N_CORES)]
    try:
        run = _get_runner()
        return run(in_maps).astype(np.float32)
    except Exception:
        nc = _get_nc()
        res = run_bass_kernel_spmd(nc, in_maps, core_ids=list(range(N_CORES)))
        return np.concatenate([res.results[i]["out"] for i in range(N_CORES)],
                              axis=0).astype(np.float32)
